# revision 35
# baseline (speedup 1.0000x reference)
"""Adaptive Computation Time step — Trainium2 Bass kernel, 8-core data-parallel.

Token layout per core: 8192 tokens -> [128 partitions, 64 columns], token
t = p*64 + c. Each column owns an H=1024 feature row of h. The h@W matvec is
a fused scalar_tensor_tensor (multiply + row-reduce accumulator) per column;
per-token mask/accumulator math is batched [128, G] on VectorE; h_out's mask
multiply runs on ScalarE. All-zero optional inputs (checked on host) skip
their DMA reads; output buffers are pre-zeroed by the runtime.
"""

import numpy as np

import concourse.bacc as bacc
import concourse.mybir as mybir
from concourse.bass_utils import run_bass_kernel_spmd
from concourse.tile import TileContext

N_CORES = 8
P = 128
THRESHOLD = 0.99

# test-harness knobs (the grading harness leaves these at defaults)
TRACE = False
TRACE_CORES = None
LAST_RESULT = None
G_COLS = 4

F32 = mybir.dt.float32
I32 = mybir.dt.int32
OP = mybir.AluOpType
ACTF = mybir.ActivationFunctionType


def build(n_tok, H, G, read_wh, read_rem, read_exit, b_val, coeff_val, step_val):
    """Build the per-core SPMD Bass graph.

    n_tok: tokens per core (multiple of 128). G: columns per DMA group.
    read_*: whether to load that input (host verified it is all-zero otherwise).
    """
    NCOL = n_tok // P
    assert NCOL % G == 0
    NG = NCOL // G

    nc = bacc.Bacc("TRN2", target_bir_lowering=False)

    h_ext = nc.declare_dram_parameter("h", [n_tok, H], F32, isOutput=False)
    wh_ext = (
        nc.declare_dram_parameter("weighted_h", [n_tok, H], F32, isOutput=False)
        if read_wh
        else None
    )
    accp_ext = nc.declare_dram_parameter("acc_p", [n_tok], F32, isOutput=False)
    rem_ext = (
        nc.declare_dram_parameter("remainders", [n_tok], F32, isOutput=False)
        if read_rem
        else None
    )
    exit_ext = (
        nc.declare_dram_parameter("exit_", [n_tok], F32, isOutput=False)
        if read_exit
        else None
    )
    run_ext = nc.declare_dram_parameter("run", [n_tok], I32, isOutput=False)
    w_ext = nc.declare_dram_parameter("W", [H], F32, isOutput=False)

    hout_ext = nc.declare_dram_parameter("h_out", [n_tok, H], F32, isOutput=True)
    whn_ext = nc.declare_dram_parameter("wh_new", [n_tok, H], F32, isOutput=True)
    accn_ext = nc.declare_dram_parameter("acc_p_new", [n_tok], F32, isOutput=True)
    remn_ext = nc.declare_dram_parameter("rem_new", [n_tok], F32, isOutput=True)
    exitn_ext = nc.declare_dram_parameter("exit_new", [n_tok], F32, isOutput=True)
    runn_ext = nc.declare_dram_parameter("run_new", [n_tok], I32, isOutput=True)

    # DRAM views: partition p <- rows p*NCOL..p*NCOL+NCOL-1 (contiguous per
    # partition, so each group DMA moves G*H*4 contiguous bytes per partition).
    hv = h_ext[:, :].rearrange("(p n) d -> p (n d)", p=P)
    whv = wh_ext[:, :].rearrange("(p n) d -> p (n d)", p=P) if read_wh else None
    hov = hout_ext[:, :].rearrange("(p n) d -> p (n d)", p=P)
    whnv = whn_ext[:, :].rearrange("(p n) d -> p (n d)", p=P)

    def small_view(ext):
        return ext[:].rearrange("(p n) -> p n", p=P)

    with TileContext(nc) as tc:
        with (
            tc.tile_pool(name="persist", bufs=1) as persist,
            tc.tile_pool(name="small", bufs=4) as small,
            tc.tile_pool(name="hin", bufs=5) as hpool,
            tc.tile_pool(name="whin", bufs=3) as whpool,
            tc.tile_pool(name="outs", bufs=2) as outpool,
            tc.tile_pool(name="scr", bufs=2) as scratch,
        ):
            # --- persistent loads / per-token precompute ([P, NCOL] each) ---
            # first h tile goes out ahead of everything so compute starts ASAP
            ht0 = hpool.tile([P, G * H], F32, tag="ht")
            nc.sync.dma_start(out=ht0[:], in_=hv[:, 0 : G * H])

            wb = persist.tile([P, H], F32)
            nc.sync.dma_start(out=wb[:1, :], in_=w_ext[:].unsqueeze(0))
            nc.gpsimd.partition_broadcast(wb[:], wb[:1, :])

            accp_sb = persist.tile([P, NCOL], F32)
            nc.sync.dma_start(out=accp_sb[:], in_=small_view(accp_ext))
            runi_sb = persist.tile([P, NCOL], I32)
            nc.sync.dma_start(out=runi_sb[:], in_=small_view(run_ext))
            if read_rem:
                rem_sb = persist.tile([P, NCOL], F32)
                nc.sync.dma_start(out=rem_sb[:], in_=small_view(rem_ext))
            else:
                rem_sb = None
            if read_exit:
                exit_sb = persist.tile([P, NCOL], F32)
                nc.sync.dma_start(out=exit_sb[:], in_=small_view(exit_ext))
            else:
                exit_sb = None

            runof = persist.tile([P, NCOL], F32)  # float(run)
            nc.vector.tensor_copy(runof[:], runi_sb[:])
            runf = persist.tile([P, NCOL], F32)  # (run > 0) as 1.0/0.0
            nc.vector.tensor_scalar(runf[:], runof[:], 0.0, None, OP.is_gt)
            runco = persist.tile([P, NCOL], F32)  # runf * coeff
            nc.vector.tensor_scalar(runco[:], runf[:], float(coeff_val), None, OP.mult)
            q0 = persist.tile([P, NCOL], F32)  # 1 - acc_p
            nc.vector.tensor_scalar(q0[:], accp_sb[:], -1.0, 1.0, OP.mult, OP.add)
            q = persist.tile([P, NCOL], F32)  # runf * (1 - acc_p)
            nc.vector.tensor_tensor(q[:], q0[:], runf[:], OP.mult)

            bias_sb = persist.tile([P, 1], F32)
            nc.vector.memset(bias_sb[:], float(b_val))

            accn_sb = persist.tile([P, NCOL], F32)
            remn_sb = persist.tile([P, NCOL], F32)
            exitn_sb = persist.tile([P, NCOL], F32)
            runnf_sb = persist.tile([P, NCOL], F32)

            # --- main loop over column groups (per-token math batched [P, G]) ---
            for g in range(NG):
                sl = slice(g * G, (g + 1) * G)
                if g == 0:
                    ht = ht0
                else:
                    ht = hpool.tile([P, G * H], F32, tag="ht")
                    nc.sync.dma_start(
                        out=ht[:], in_=hv[:, g * G * H : (g + 1) * G * H]
                    )
                if read_wh:
                    wt = whpool.tile([P, G * H], F32)
                    nc.sync.dma_start(
                        out=wt[:], in_=whv[:, g * G * H : (g + 1) * G * H]
                    )
                hot = outpool.tile([P, G * H], F32)
                whnt = outpool.tile([P, G * H], F32, tag="whnt")

                hwg = small.tile([P, G], F32)
                for j in range(G):
                    sc = scratch.tile([P, H], F32)
                    # hw = sum_d h[:,d] * W[d]  (fused multiply + row-reduce)
                    nc.vector.scalar_tensor_tensor(
                        sc[:], ht[:, j * H : (j + 1) * H], 1.0, wb[:],
                        OP.mult, OP.mult, accum_out=hwg[:, j : j + 1],
                    )
                p0g = small.tile([P, G], F32)
                nc.scalar.activation(
                    p0g[:], hwg[:], ACTF.Sigmoid, bias=bias_sb[:], scale=1.0
                )
                pg = small.tile([P, G], F32)  # p = sigmoid * coeff * runf
                nc.vector.tensor_tensor(pg[:], p0g[:], runco[:, sl], OP.mult)
                sg = small.tile([P, G], F32)  # acc_p + p
                nc.vector.tensor_tensor(sg[:], pg[:], accp_sb[:, sl], OP.add)
                mcg = small.tile([P, G], F32)  # (s < T) * runf
                nc.vector.scalar_tensor_tensor(
                    mcg[:], sg[:], THRESHOLD, runf[:, sl], OP.is_lt, OP.mult
                )
                # run_new = run * mc — FIRST after mcg: it gates ScalarE's
                # h_out muls, the tail of the per-group critical path
                nc.vector.tensor_tensor(runnf_sb[:, sl], mcg[:], runof[:, sl], OP.mult)
                meg = small.tile([P, G], F32)  # runf - mc
                nc.vector.tensor_tensor(meg[:], runf[:, sl], mcg[:], OP.subtract)
                t2g = small.tile([P, G], F32)  # me * (1-acc_p)  (masked)
                nc.vector.tensor_tensor(t2g[:], meg[:], q[:, sl], OP.mult)
                u1g = small.tile([P, G], F32)  # p * mc
                nc.vector.tensor_tensor(u1g[:], mcg[:], pg[:], OP.mult)
                updg = small.tile([P, G], F32)  # p*mc + (1-acc_p)*me
                nc.vector.tensor_tensor(updg[:], u1g[:], t2g[:], OP.add)
                # small output columns — off the critical path, emitted last
                # acc_p_new = acc_p + p*mc
                nc.vector.tensor_tensor(accn_sb[:, sl], u1g[:], accp_sb[:, sl], OP.add)
                # remainders_new = remainders + (1-acc_p)*me
                if read_rem:
                    nc.vector.tensor_tensor(
                        remn_sb[:, sl], t2g[:], rem_sb[:, sl], OP.add
                    )
                else:
                    nc.vector.tensor_copy(remn_sb[:, sl], t2g[:])
                # exit_new = exit_ + step*me
                if read_exit:
                    u2g = small.tile([P, G], F32)
                    nc.vector.tensor_scalar(
                        u2g[:], meg[:], float(step_val), None, OP.mult
                    )
                    nc.vector.tensor_tensor(
                        exitn_sb[:, sl], u2g[:], exit_sb[:, sl], OP.add
                    )
                else:
                    nc.vector.tensor_scalar(
                        exitn_sb[:, sl], meg[:], float(step_val), None, OP.mult
                    )

                for j in range(G):
                    col = g * G + j
                    hs = ht[:, j * H : (j + 1) * H]
                    # weighted_h_new = h*update (+ weighted_h)
                    ws = whnt[:, j * H : (j + 1) * H]
                    if read_wh:
                        nc.vector.scalar_tensor_tensor(
                            ws, hs, updg[:, j : j + 1], wt[:, j * H : (j + 1) * H],
                            OP.mult, OP.add,
                        )
                    else:
                        nc.vector.tensor_scalar(
                            ws, hs, updg[:, j : j + 1], None, OP.mult
                        )
                    # h_out = h * run_new  (ScalarE: per-partition scale)
                    nc.scalar.mul(
                        hot[:, j * H : (j + 1) * H], hs, runnf_sb[:, col : col + 1]
                    )

                nc.sync.dma_start(
                    out=hov[:, g * G * H : (g + 1) * G * H], in_=hot[:]
                )
                nc.sync.dma_start(
                    out=whnv[:, g * G * H : (g + 1) * G * H], in_=whnt[:]
                )

            # --- tail: small outputs ---
            runni = persist.tile([P, NCOL], I32)
            nc.vector.tensor_copy(runni[:], runnf_sb[:])
            nc.sync.dma_start(out=small_view(accn_ext), in_=accn_sb[:])
            nc.sync.dma_start(out=small_view(remn_ext), in_=remn_sb[:])
            nc.sync.dma_start(out=small_view(exitn_ext), in_=exitn_sb[:])
            nc.sync.dma_start(out=small_view(runn_ext), in_=runni[:])

    nc.finalize()
    return nc


def kernel(h, weighted_h, acc_p, remainders, exit_, run, W, b, coeff, step):
    h = np.asarray(h, dtype=np.float32)
    weighted_h = np.asarray(weighted_h, dtype=np.float32)
    acc_p = np.asarray(acc_p, dtype=np.float32)
    remainders = np.asarray(remainders, dtype=np.float32)
    exit_ = np.asarray(exit_, dtype=np.float32)
    run = np.asarray(run, dtype=np.int32)
    W = np.asarray(W, dtype=np.float32)
    b_val = float(np.asarray(b, dtype=np.float32).ravel()[0])
    coeff_val = float(np.asarray(coeff, dtype=np.float32).ravel()[0])
    step_val = float(np.asarray(step).ravel()[0])

    B, M, H = h.shape
    n_tok = B * M
    per = n_tok // N_CORES
    read_wh = bool(weighted_h.any())
    read_rem = bool(remainders.any())
    read_exit = bool(exit_.any())

    ncol = per // P
    g = G_COLS if ncol % G_COLS == 0 else (2 if ncol % 2 == 0 else 1)
    nc = build(per, H, g, read_wh, read_rem, read_exit, b_val, coeff_val, step_val)

    hf = h.reshape(n_tok, H)
    whf = weighted_h.reshape(n_tok, H)
    accf = np.ascontiguousarray(acc_p.reshape(n_tok))
    remf = np.ascontiguousarray(remainders.reshape(n_tok))
    exitf = np.ascontiguousarray(exit_.reshape(n_tok))
    runi = np.ascontiguousarray(run.reshape(n_tok))
    wf = np.ascontiguousarray(W.reshape(H))

    in_maps = []
    for c in range(N_CORES):
        s = slice(c * per, (c + 1) * per)
        m = {"h": hf[s], "acc_p": accf[s], "run": runi[s], "W": wf}
        if read_wh:
            m["weighted_h"] = whf[s]
        if read_rem:
            m["remainders"] = remf[s]
        if read_exit:
            m["exit_"] = exitf[s]
        in_maps.append(m)

    global LAST_RESULT
    LAST_RESULT = run_bass_kernel_spmd(
        nc, in_maps, list(range(N_CORES)), trace=TRACE, trace_cores=TRACE_CORES
    )
    results = LAST_RESULT.results

    h_out = np.concatenate([r["h_out"] for r in results]).reshape(B, M, H)
    wh_new = np.concatenate([r["wh_new"] for r in results]).reshape(B, M, H)
    acc_p_new = np.concatenate([r["acc_p_new"] for r in results]).reshape(B, M, 1)
    rem_new = np.concatenate([r["rem_new"] for r in results]).reshape(B, M, 1)
    exit_new = np.concatenate([r["exit_new"] for r in results]).reshape(B, M, 1)
    run_new = np.concatenate([r["run_new"] for r in results]).reshape(B, M, 1)
    return (h_out, wh_new, acc_p_new, rem_new, exit_new, run_new)


# revision 41
# speedup vs baseline: 1.1272x; 1.1272x over previous
"""Adaptive Computation Time step — Trainium2 Bass kernel, 8-core data-parallel.

Token layout per core: 8192 tokens -> [128 partitions, 64 columns], token
t = p*64 + c. Each column owns an H=1024 feature row of h. The h@W matvec is
a fused scalar_tensor_tensor (multiply + row-reduce accumulator) per column;
per-token mask/accumulator math is batched [128, G] on VectorE; h_out's mask
multiply runs on ScalarE. All-zero optional inputs (checked on host) skip
their DMA reads; output buffers are pre-zeroed by the runtime.
"""

import numpy as np

import concourse.bacc as bacc
import concourse.mybir as mybir
from concourse.bass_utils import run_bass_kernel_spmd
from concourse.tile import TileContext

N_CORES = 8
P = 128
THRESHOLD = 0.99

# test-harness knobs (the grading harness leaves these at defaults)
TRACE = False
TRACE_CORES = None
LAST_RESULT = None
G_COLS = 4

F32 = mybir.dt.float32
I32 = mybir.dt.int32
OP = mybir.AluOpType
ACTF = mybir.ActivationFunctionType


def build(n_tok, H, G, read_wh, read_rem, read_exit, b_val, coeff_val, step_val):
    """Build the per-core SPMD Bass graph.

    n_tok: tokens per core (multiple of 128). G: columns per DMA group.
    read_*: whether to load that input (host verified it is all-zero otherwise).
    """
    NCOL = n_tok // P
    assert NCOL % G == 0
    NG = NCOL // G

    nc = bacc.Bacc("TRN2", target_bir_lowering=False)

    h_ext = nc.declare_dram_parameter("h", [n_tok, H], F32, isOutput=False)
    wh_ext = (
        nc.declare_dram_parameter("weighted_h", [n_tok, H], F32, isOutput=False)
        if read_wh
        else None
    )
    accp_ext = nc.declare_dram_parameter("acc_p", [n_tok], F32, isOutput=False)
    rem_ext = (
        nc.declare_dram_parameter("remainders", [n_tok], F32, isOutput=False)
        if read_rem
        else None
    )
    exit_ext = (
        nc.declare_dram_parameter("exit_", [n_tok], F32, isOutput=False)
        if read_exit
        else None
    )
    run_ext = nc.declare_dram_parameter("run", [n_tok], I32, isOutput=False)
    w_ext = nc.declare_dram_parameter("W", [H], F32, isOutput=False)

    hout_ext = nc.declare_dram_parameter("h_out", [n_tok, H], F32, isOutput=True)
    whn_ext = nc.declare_dram_parameter("wh_new", [n_tok, H], F32, isOutput=True)
    accn_ext = nc.declare_dram_parameter("acc_p_new", [n_tok], F32, isOutput=True)
    remn_ext = nc.declare_dram_parameter("rem_new", [n_tok], F32, isOutput=True)
    exitn_ext = nc.declare_dram_parameter("exit_new", [n_tok], F32, isOutput=True)
    runn_ext = nc.declare_dram_parameter("run_new", [n_tok], I32, isOutput=True)

    # DRAM views: partition p <- rows p*NCOL..p*NCOL+NCOL-1 (contiguous per
    # partition, so each group DMA moves G*H*4 contiguous bytes per partition).
    hv = h_ext[:, :].rearrange("(p n) d -> p (n d)", p=P)
    whv = wh_ext[:, :].rearrange("(p n) d -> p (n d)", p=P) if read_wh else None
    hov = hout_ext[:, :].rearrange("(p n) d -> p (n d)", p=P)
    whnv = whn_ext[:, :].rearrange("(p n) d -> p (n d)", p=P)

    def small_view(ext):
        return ext[:].rearrange("(p n) -> p n", p=P)

    with TileContext(nc) as tc:
        with (
            tc.tile_pool(name="persist", bufs=1) as persist,
            tc.tile_pool(name="small", bufs=4) as small,
            tc.tile_pool(name="hin", bufs=4) as hpool,
            tc.tile_pool(name="whin", bufs=3) as whpool,
            tc.tile_pool(name="outs", bufs=2) as outpool,
            tc.tile_pool(name="scr", bufs=2) as scratch,
        ):
            # --- persistent loads / per-token precompute ([P, NCOL] each) ---
            # first h tile goes out ahead of everything so compute starts ASAP
            ht0 = hpool.tile([P, G * H], F32, tag="ht")
            nc.sync.dma_start(out=ht0[:], in_=hv[:, 0 : G * H])

            wb = persist.tile([P, H], F32)
            nc.sync.dma_start(out=wb[:1, :], in_=w_ext[:].unsqueeze(0))
            nc.gpsimd.partition_broadcast(wb[:], wb[:1, :])

            accp_sb = persist.tile([P, NCOL], F32)
            nc.sync.dma_start(out=accp_sb[:], in_=small_view(accp_ext))
            runi_sb = persist.tile([P, NCOL], I32)
            nc.sync.dma_start(out=runi_sb[:], in_=small_view(run_ext))
            if read_rem:
                rem_sb = persist.tile([P, NCOL], F32)
                nc.sync.dma_start(out=rem_sb[:], in_=small_view(rem_ext))
            else:
                rem_sb = None
            if read_exit:
                exit_sb = persist.tile([P, NCOL], F32)
                nc.sync.dma_start(out=exit_sb[:], in_=small_view(exit_ext))
            else:
                exit_sb = None

            runof = persist.tile([P, NCOL], F32)  # float(run)
            nc.vector.tensor_copy(runof[:], runi_sb[:])
            runf = persist.tile([P, NCOL], F32)  # (run > 0) as 1.0/0.0
            nc.vector.tensor_scalar(runf[:], runof[:], 0.0, None, OP.is_gt)
            runco = persist.tile([P, NCOL], F32)  # runf * coeff
            nc.vector.tensor_scalar(runco[:], runf[:], float(coeff_val), None, OP.mult)
            q0 = persist.tile([P, NCOL], F32)  # 1 - acc_p
            nc.vector.tensor_scalar(q0[:], accp_sb[:], -1.0, 1.0, OP.mult, OP.add)
            q = persist.tile([P, NCOL], F32)  # runf * (1 - acc_p)
            nc.vector.tensor_tensor(q[:], q0[:], runf[:], OP.mult)

            bias_sb = persist.tile([P, 1], F32)
            nc.vector.memset(bias_sb[:], float(b_val))

            accn_sb = persist.tile([P, NCOL], F32)
            remn_sb = persist.tile([P, NCOL], F32)
            exitn_sb = persist.tile([P, NCOL], F32)
            runnf_sb = persist.tile([P, NCOL], F32)

            if read_wh:
                wt0 = whpool.tile([P, G * H], F32, tag="wt")
                nc.sync.dma_start(out=wt0[:], in_=whv[:, 0 : G * H])
            else:
                wt0 = None

            # --- main loop over column groups (per-token math batched [P, G]).
            # Input DMAs for group g+1 are issued BEFORE group g's output DMAs:
            # the sync engine's stream is in-order, and the output DMAs wait on
            # compute — issuing them first would stall input prefetch behind
            # them (head-of-line blocking on the queue).
            ht, wt = ht0, wt0
            for g in range(NG):
                sl = slice(g * G, (g + 1) * G)
                hot = outpool.tile([P, G * H], F32)
                whnt = outpool.tile([P, G * H], F32, tag="whnt")

                hwg = small.tile([P, G], F32)
                for j in range(G):
                    sc = scratch.tile([P, H], F32)
                    # hw = sum_d h[:,d] * W[d]  (fused multiply + row-reduce)
                    nc.vector.scalar_tensor_tensor(
                        sc[:], ht[:, j * H : (j + 1) * H], 1.0, wb[:],
                        OP.mult, OP.mult, accum_out=hwg[:, j : j + 1],
                    )
                p0g = small.tile([P, G], F32)
                nc.scalar.activation(
                    p0g[:], hwg[:], ACTF.Sigmoid, bias=bias_sb[:], scale=1.0
                )
                pg = small.tile([P, G], F32)  # p = sigmoid * coeff * runf
                nc.vector.tensor_tensor(pg[:], p0g[:], runco[:, sl], OP.mult)
                sg = small.tile([P, G], F32)  # acc_p + p
                nc.vector.tensor_tensor(sg[:], pg[:], accp_sb[:, sl], OP.add)
                mcg = small.tile([P, G], F32)  # (s < T) * runf
                nc.vector.scalar_tensor_tensor(
                    mcg[:], sg[:], THRESHOLD, runf[:, sl], OP.is_lt, OP.mult
                )
                meg = small.tile([P, G], F32)  # runf - mc
                nc.vector.tensor_tensor(meg[:], runf[:, sl], mcg[:], OP.subtract)
                t2g = small.tile([P, G], F32)  # me * (1-acc_p)  (masked)
                nc.vector.tensor_tensor(t2g[:], meg[:], q[:, sl], OP.mult)
                u1g = small.tile([P, G], F32)  # p * mc
                nc.vector.tensor_tensor(u1g[:], mcg[:], pg[:], OP.mult)
                updg = small.tile([P, G], F32)  # p*mc + (1-acc_p)*me
                nc.vector.tensor_tensor(updg[:], u1g[:], t2g[:], OP.add)
                # acc_p_new = acc_p + p*mc
                nc.vector.tensor_tensor(accn_sb[:, sl], u1g[:], accp_sb[:, sl], OP.add)
                # remainders_new = remainders + (1-acc_p)*me
                if read_rem:
                    nc.vector.tensor_tensor(
                        remn_sb[:, sl], t2g[:], rem_sb[:, sl], OP.add
                    )
                else:
                    nc.vector.tensor_copy(remn_sb[:, sl], t2g[:])
                # exit_new = exit_ + step*me
                if read_exit:
                    u2g = small.tile([P, G], F32)
                    nc.vector.tensor_scalar(
                        u2g[:], meg[:], float(step_val), None, OP.mult
                    )
                    nc.vector.tensor_tensor(
                        exitn_sb[:, sl], u2g[:], exit_sb[:, sl], OP.add
                    )
                else:
                    nc.vector.tensor_scalar(
                        exitn_sb[:, sl], meg[:], float(step_val), None, OP.mult
                    )
                # run_new = run * mc (as float; cast at the end)
                nc.vector.tensor_tensor(runnf_sb[:, sl], mcg[:], runof[:, sl], OP.mult)

                for j in range(G):
                    col = g * G + j
                    hs = ht[:, j * H : (j + 1) * H]
                    # weighted_h_new = h*update (+ weighted_h)
                    ws = whnt[:, j * H : (j + 1) * H]
                    if read_wh:
                        nc.vector.scalar_tensor_tensor(
                            ws, hs, updg[:, j : j + 1], wt[:, j * H : (j + 1) * H],
                            OP.mult, OP.add,
                        )
                    else:
                        nc.vector.tensor_scalar(
                            ws, hs, updg[:, j : j + 1], None, OP.mult
                        )
                    # h_out = h * run_new  (ScalarE: per-partition scale)
                    nc.scalar.mul(
                        hot[:, j * H : (j + 1) * H], hs, runnf_sb[:, col : col + 1]
                    )

                ht_next, wt_next = None, None
                if g + 1 < NG:
                    ht_next = hpool.tile([P, G * H], F32, tag="ht")
                    nc.sync.dma_start(
                        out=ht_next[:],
                        in_=hv[:, (g + 1) * G * H : (g + 2) * G * H],
                    )
                    if read_wh:
                        wt_next = whpool.tile([P, G * H], F32, tag="wt")
                        nc.sync.dma_start(
                            out=wt_next[:],
                            in_=whv[:, (g + 1) * G * H : (g + 2) * G * H],
                        )
                # whnt (VectorE, finishes earlier) ahead of hot (ScalarE)
                nc.sync.dma_start(
                    out=whnv[:, g * G * H : (g + 1) * G * H], in_=whnt[:]
                )
                nc.sync.dma_start(
                    out=hov[:, g * G * H : (g + 1) * G * H], in_=hot[:]
                )
                ht, wt = ht_next, wt_next

            # --- tail: small outputs ---
            runni = persist.tile([P, NCOL], I32)
            nc.vector.tensor_copy(runni[:], runnf_sb[:])
            nc.sync.dma_start(out=small_view(accn_ext), in_=accn_sb[:])
            nc.sync.dma_start(out=small_view(remn_ext), in_=remn_sb[:])
            nc.sync.dma_start(out=small_view(exitn_ext), in_=exitn_sb[:])
            nc.sync.dma_start(out=small_view(runn_ext), in_=runni[:])

    nc.finalize()
    return nc


def kernel(h, weighted_h, acc_p, remainders, exit_, run, W, b, coeff, step):
    h = np.asarray(h, dtype=np.float32)
    weighted_h = np.asarray(weighted_h, dtype=np.float32)
    acc_p = np.asarray(acc_p, dtype=np.float32)
    remainders = np.asarray(remainders, dtype=np.float32)
    exit_ = np.asarray(exit_, dtype=np.float32)
    run = np.asarray(run, dtype=np.int32)
    W = np.asarray(W, dtype=np.float32)
    b_val = float(np.asarray(b, dtype=np.float32).ravel()[0])
    coeff_val = float(np.asarray(coeff, dtype=np.float32).ravel()[0])
    step_val = float(np.asarray(step).ravel()[0])

    B, M, H = h.shape
    n_tok = B * M
    per = n_tok // N_CORES
    read_wh = bool(weighted_h.any())
    read_rem = bool(remainders.any())
    read_exit = bool(exit_.any())

    ncol = per // P
    g = G_COLS if ncol % G_COLS == 0 else (2 if ncol % 2 == 0 else 1)
    nc = build(per, H, g, read_wh, read_rem, read_exit, b_val, coeff_val, step_val)

    hf = h.reshape(n_tok, H)
    whf = weighted_h.reshape(n_tok, H)
    accf = np.ascontiguousarray(acc_p.reshape(n_tok))
    remf = np.ascontiguousarray(remainders.reshape(n_tok))
    exitf = np.ascontiguousarray(exit_.reshape(n_tok))
    runi = np.ascontiguousarray(run.reshape(n_tok))
    wf = np.ascontiguousarray(W.reshape(H))

    in_maps = []
    for c in range(N_CORES):
        s = slice(c * per, (c + 1) * per)
        m = {"h": hf[s], "acc_p": accf[s], "run": runi[s], "W": wf}
        if read_wh:
            m["weighted_h"] = whf[s]
        if read_rem:
            m["remainders"] = remf[s]
        if read_exit:
            m["exit_"] = exitf[s]
        in_maps.append(m)

    global LAST_RESULT
    LAST_RESULT = run_bass_kernel_spmd(
        nc, in_maps, list(range(N_CORES)), trace=TRACE, trace_cores=TRACE_CORES
    )
    results = LAST_RESULT.results

    h_out = np.concatenate([r["h_out"] for r in results]).reshape(B, M, H)
    wh_new = np.concatenate([r["wh_new"] for r in results]).reshape(B, M, H)
    acc_p_new = np.concatenate([r["acc_p_new"] for r in results]).reshape(B, M, 1)
    rem_new = np.concatenate([r["rem_new"] for r in results]).reshape(B, M, 1)
    exit_new = np.concatenate([r["exit_new"] for r in results]).reshape(B, M, 1)
    run_new = np.concatenate([r["run_new"] for r in results]).reshape(B, M, 1)
    return (h_out, wh_new, acc_p_new, rem_new, exit_new, run_new)


# revision 43
# speedup vs baseline: 1.1637x; 1.0324x over previous
"""Adaptive Computation Time step — Trainium2 Bass kernel, 8-core data-parallel.

Token layout per core: 8192 tokens -> [128 partitions, 64 columns], token
t = p*64 + c. Each column owns an H=1024 feature row of h. The h@W matvec is
a fused scalar_tensor_tensor (multiply + row-reduce accumulator) per column;
per-token mask/accumulator math is batched [128, G] on VectorE; h_out's mask
multiply runs on ScalarE. All-zero optional inputs (checked on host) skip
their DMA reads; output buffers are pre-zeroed by the runtime.
"""

import numpy as np

import concourse.bacc as bacc
import concourse.mybir as mybir
from concourse.bass_utils import run_bass_kernel_spmd
from concourse.tile import TileContext

N_CORES = 8
P = 128
THRESHOLD = 0.99

# test-harness knobs (the grading harness leaves these at defaults)
TRACE = False
TRACE_CORES = None
LAST_RESULT = None
G_COLS = 4

F32 = mybir.dt.float32
I32 = mybir.dt.int32
OP = mybir.AluOpType
ACTF = mybir.ActivationFunctionType


def build(n_tok, H, G, read_wh, read_rem, read_exit, b_val, coeff_val, step_val):
    """Build the per-core SPMD Bass graph.

    n_tok: tokens per core (multiple of 128). G: columns per DMA group.
    read_*: whether to load that input (host verified it is all-zero otherwise).
    """
    NCOL = n_tok // P
    assert NCOL % G == 0
    NG = NCOL // G

    nc = bacc.Bacc("TRN2", target_bir_lowering=False)

    h_ext = nc.declare_dram_parameter("h", [n_tok, H], F32, isOutput=False)
    wh_ext = (
        nc.declare_dram_parameter("weighted_h", [n_tok, H], F32, isOutput=False)
        if read_wh
        else None
    )
    accp_ext = nc.declare_dram_parameter("acc_p", [n_tok], F32, isOutput=False)
    rem_ext = (
        nc.declare_dram_parameter("remainders", [n_tok], F32, isOutput=False)
        if read_rem
        else None
    )
    exit_ext = (
        nc.declare_dram_parameter("exit_", [n_tok], F32, isOutput=False)
        if read_exit
        else None
    )
    run_ext = nc.declare_dram_parameter("run", [n_tok], I32, isOutput=False)
    w_ext = nc.declare_dram_parameter("W", [H], F32, isOutput=False)

    hout_ext = nc.declare_dram_parameter("h_out", [n_tok, H], F32, isOutput=True)
    whn_ext = nc.declare_dram_parameter("wh_new", [n_tok, H], F32, isOutput=True)
    accn_ext = nc.declare_dram_parameter("acc_p_new", [n_tok], F32, isOutput=True)
    remn_ext = nc.declare_dram_parameter("rem_new", [n_tok], F32, isOutput=True)
    exitn_ext = nc.declare_dram_parameter("exit_new", [n_tok], F32, isOutput=True)
    runn_ext = nc.declare_dram_parameter("run_new", [n_tok], I32, isOutput=True)

    # DRAM views: partition p <- rows p*NCOL..p*NCOL+NCOL-1 (contiguous per
    # partition, so each group DMA moves G*H*4 contiguous bytes per partition).
    hv = h_ext[:, :].rearrange("(p n) d -> p (n d)", p=P)
    whv = wh_ext[:, :].rearrange("(p n) d -> p (n d)", p=P) if read_wh else None
    hov = hout_ext[:, :].rearrange("(p n) d -> p (n d)", p=P)
    whnv = whn_ext[:, :].rearrange("(p n) d -> p (n d)", p=P)

    def small_view(ext):
        return ext[:].rearrange("(p n) -> p n", p=P)

    with TileContext(nc) as tc:
        with (
            tc.tile_pool(name="persist", bufs=1) as persist,
            tc.tile_pool(name="small", bufs=4) as small,
            tc.tile_pool(name="hin", bufs=4) as hpool,
            tc.tile_pool(name="whin", bufs=3) as whpool,
            tc.tile_pool(name="outs", bufs=2) as outpool,
            tc.tile_pool(name="scr", bufs=2) as scratch,
        ):
            # --- persistent loads / per-token precompute ([P, NCOL] each) ---
            # W first (4KB, needed by the first matvec), then the first h tile
            # in halves so compute starts after 1MB instead of 2MB
            wb = persist.tile([P, H], F32)
            nc.sync.dma_start(out=wb[:1, :], in_=w_ext[:].unsqueeze(0))
            nc.gpsimd.partition_broadcast(wb[:], wb[:1, :])

            GH2 = (G * H) // 2
            ht0 = hpool.tile([P, G * H], F32, tag="ht")
            nc.sync.dma_start(out=ht0[:, :GH2], in_=hv[:, 0:GH2])
            nc.sync.dma_start(out=ht0[:, GH2:], in_=hv[:, GH2 : G * H])

            accp_sb = persist.tile([P, NCOL], F32)
            nc.sync.dma_start(out=accp_sb[:], in_=small_view(accp_ext))
            runi_sb = persist.tile([P, NCOL], I32)
            nc.sync.dma_start(out=runi_sb[:], in_=small_view(run_ext))
            if read_rem:
                rem_sb = persist.tile([P, NCOL], F32)
                nc.sync.dma_start(out=rem_sb[:], in_=small_view(rem_ext))
            else:
                rem_sb = None
            if read_exit:
                exit_sb = persist.tile([P, NCOL], F32)
                nc.sync.dma_start(out=exit_sb[:], in_=small_view(exit_ext))
            else:
                exit_sb = None

            runof = persist.tile([P, NCOL], F32)  # float(run)
            nc.vector.tensor_copy(runof[:], runi_sb[:])
            runf = persist.tile([P, NCOL], F32)  # (run > 0) as 1.0/0.0
            nc.vector.tensor_scalar(runf[:], runof[:], 0.0, None, OP.is_gt)
            runco = persist.tile([P, NCOL], F32)  # runf * coeff
            nc.vector.tensor_scalar(runco[:], runf[:], float(coeff_val), None, OP.mult)
            q0 = persist.tile([P, NCOL], F32)  # 1 - acc_p
            nc.vector.tensor_scalar(q0[:], accp_sb[:], -1.0, 1.0, OP.mult, OP.add)
            q = persist.tile([P, NCOL], F32)  # runf * (1 - acc_p)
            nc.vector.tensor_tensor(q[:], q0[:], runf[:], OP.mult)

            bias_sb = persist.tile([P, 1], F32)
            nc.vector.memset(bias_sb[:], float(b_val))

            accn_sb = persist.tile([P, NCOL], F32)
            remn_sb = persist.tile([P, NCOL], F32)
            exitn_sb = persist.tile([P, NCOL], F32)
            runnf_sb = persist.tile([P, NCOL], F32)

            if read_wh:
                wt0 = whpool.tile([P, G * H], F32, tag="wt")
                nc.sync.dma_start(out=wt0[:], in_=whv[:, 0 : G * H])
            else:
                wt0 = None

            # --- main loop over column groups (per-token math batched [P, G]).
            # Input DMAs for group g+1 are issued BEFORE group g's output DMAs:
            # the sync engine's stream is in-order, and the output DMAs wait on
            # compute — issuing them first would stall input prefetch behind
            # them (head-of-line blocking on the queue).
            ht, wt = ht0, wt0
            for g in range(NG):
                sl = slice(g * G, (g + 1) * G)
                hot = outpool.tile([P, G * H], F32)
                whnt = outpool.tile([P, G * H], F32, tag="whnt")

                hwg = small.tile([P, G], F32)
                for j in range(G):
                    sc = scratch.tile([P, H], F32)
                    # hw = sum_d h[:,d] * W[d]  (fused multiply + row-reduce)
                    nc.vector.scalar_tensor_tensor(
                        sc[:], ht[:, j * H : (j + 1) * H], 1.0, wb[:],
                        OP.mult, OP.mult, accum_out=hwg[:, j : j + 1],
                    )
                p0g = small.tile([P, G], F32)
                nc.scalar.activation(
                    p0g[:], hwg[:], ACTF.Sigmoid, bias=bias_sb[:], scale=1.0
                )
                pg = small.tile([P, G], F32)  # p = sigmoid * coeff * runf
                nc.vector.tensor_tensor(pg[:], p0g[:], runco[:, sl], OP.mult)
                sg = small.tile([P, G], F32)  # acc_p + p
                nc.vector.tensor_tensor(sg[:], pg[:], accp_sb[:, sl], OP.add)
                mcg = small.tile([P, G], F32)  # (s < T) * runf
                nc.vector.scalar_tensor_tensor(
                    mcg[:], sg[:], THRESHOLD, runf[:, sl], OP.is_lt, OP.mult
                )
                meg = small.tile([P, G], F32)  # runf - mc
                nc.vector.tensor_tensor(meg[:], runf[:, sl], mcg[:], OP.subtract)
                t2g = small.tile([P, G], F32)  # me * (1-acc_p)  (masked)
                nc.vector.tensor_tensor(t2g[:], meg[:], q[:, sl], OP.mult)
                u1g = small.tile([P, G], F32)  # p * mc
                nc.vector.tensor_tensor(u1g[:], mcg[:], pg[:], OP.mult)
                updg = small.tile([P, G], F32)  # p*mc + (1-acc_p)*me
                nc.vector.tensor_tensor(updg[:], u1g[:], t2g[:], OP.add)
                # acc_p_new = acc_p + p*mc
                nc.vector.tensor_tensor(accn_sb[:, sl], u1g[:], accp_sb[:, sl], OP.add)
                # remainders_new = remainders + (1-acc_p)*me
                if read_rem:
                    nc.vector.tensor_tensor(
                        remn_sb[:, sl], t2g[:], rem_sb[:, sl], OP.add
                    )
                else:
                    nc.vector.tensor_copy(remn_sb[:, sl], t2g[:])
                # exit_new = exit_ + step*me
                if read_exit:
                    u2g = small.tile([P, G], F32)
                    nc.vector.tensor_scalar(
                        u2g[:], meg[:], float(step_val), None, OP.mult
                    )
                    nc.vector.tensor_tensor(
                        exitn_sb[:, sl], u2g[:], exit_sb[:, sl], OP.add
                    )
                else:
                    nc.vector.tensor_scalar(
                        exitn_sb[:, sl], meg[:], float(step_val), None, OP.mult
                    )
                # run_new = run * mc (as float; cast at the end)
                nc.vector.tensor_tensor(runnf_sb[:, sl], mcg[:], runof[:, sl], OP.mult)

                for j in range(G):
                    col = g * G + j
                    hs = ht[:, j * H : (j + 1) * H]
                    # weighted_h_new = h*update (+ weighted_h)
                    ws = whnt[:, j * H : (j + 1) * H]
                    if read_wh:
                        nc.vector.scalar_tensor_tensor(
                            ws, hs, updg[:, j : j + 1], wt[:, j * H : (j + 1) * H],
                            OP.mult, OP.add,
                        )
                    else:
                        nc.vector.tensor_scalar(
                            ws, hs, updg[:, j : j + 1], None, OP.mult
                        )
                    # h_out = h * run_new  (ScalarE: per-partition scale)
                    nc.scalar.mul(
                        hot[:, j * H : (j + 1) * H], hs, runnf_sb[:, col : col + 1]
                    )

                ht_next, wt_next = None, None
                if g + 1 < NG:
                    ht_next = hpool.tile([P, G * H], F32, tag="ht")
                    nc.sync.dma_start(
                        out=ht_next[:],
                        in_=hv[:, (g + 1) * G * H : (g + 2) * G * H],
                    )
                    if read_wh:
                        wt_next = whpool.tile([P, G * H], F32, tag="wt")
                        nc.sync.dma_start(
                            out=wt_next[:],
                            in_=whv[:, (g + 1) * G * H : (g + 2) * G * H],
                        )
                # whnt (VectorE, finishes earlier) ahead of hot (ScalarE)
                base = g * G * H
                if g + 1 < NG:
                    nc.sync.dma_start(out=whnv[:, base : base + G * H], in_=whnt[:])
                    nc.sync.dma_start(out=hov[:, base : base + G * H], in_=hot[:])
                else:
                    # last group: halves, interleaved by readiness, so the
                    # final drain starts as soon as the first columns finish
                    nc.sync.dma_start(
                        out=whnv[:, base : base + GH2], in_=whnt[:, :GH2]
                    )
                    nc.sync.dma_start(out=hov[:, base : base + GH2], in_=hot[:, :GH2])
                    nc.sync.dma_start(
                        out=whnv[:, base + GH2 : base + G * H], in_=whnt[:, GH2:]
                    )
                    nc.sync.dma_start(
                        out=hov[:, base + GH2 : base + G * H], in_=hot[:, GH2:]
                    )
                ht, wt = ht_next, wt_next

            # --- tail: small outputs ---
            runni = persist.tile([P, NCOL], I32)
            nc.vector.tensor_copy(runni[:], runnf_sb[:])
            nc.sync.dma_start(out=small_view(accn_ext), in_=accn_sb[:])
            nc.sync.dma_start(out=small_view(remn_ext), in_=remn_sb[:])
            nc.sync.dma_start(out=small_view(exitn_ext), in_=exitn_sb[:])
            nc.sync.dma_start(out=small_view(runn_ext), in_=runni[:])

    nc.finalize()
    return nc


def kernel(h, weighted_h, acc_p, remainders, exit_, run, W, b, coeff, step):
    h = np.asarray(h, dtype=np.float32)
    weighted_h = np.asarray(weighted_h, dtype=np.float32)
    acc_p = np.asarray(acc_p, dtype=np.float32)
    remainders = np.asarray(remainders, dtype=np.float32)
    exit_ = np.asarray(exit_, dtype=np.float32)
    run = np.asarray(run, dtype=np.int32)
    W = np.asarray(W, dtype=np.float32)
    b_val = float(np.asarray(b, dtype=np.float32).ravel()[0])
    coeff_val = float(np.asarray(coeff, dtype=np.float32).ravel()[0])
    step_val = float(np.asarray(step).ravel()[0])

    B, M, H = h.shape
    n_tok = B * M
    per = n_tok // N_CORES
    read_wh = bool(weighted_h.any())
    read_rem = bool(remainders.any())
    read_exit = bool(exit_.any())

    ncol = per // P
    g = G_COLS if ncol % G_COLS == 0 else (2 if ncol % 2 == 0 else 1)
    nc = build(per, H, g, read_wh, read_rem, read_exit, b_val, coeff_val, step_val)

    hf = h.reshape(n_tok, H)
    whf = weighted_h.reshape(n_tok, H)
    accf = np.ascontiguousarray(acc_p.reshape(n_tok))
    remf = np.ascontiguousarray(remainders.reshape(n_tok))
    exitf = np.ascontiguousarray(exit_.reshape(n_tok))
    runi = np.ascontiguousarray(run.reshape(n_tok))
    wf = np.ascontiguousarray(W.reshape(H))

    in_maps = []
    for c in range(N_CORES):
        s = slice(c * per, (c + 1) * per)
        m = {"h": hf[s], "acc_p": accf[s], "run": runi[s], "W": wf}
        if read_wh:
            m["weighted_h"] = whf[s]
        if read_rem:
            m["remainders"] = remf[s]
        if read_exit:
            m["exit_"] = exitf[s]
        in_maps.append(m)

    global LAST_RESULT
    LAST_RESULT = run_bass_kernel_spmd(
        nc, in_maps, list(range(N_CORES)), trace=TRACE, trace_cores=TRACE_CORES
    )
    results = LAST_RESULT.results

    h_out = np.concatenate([r["h_out"] for r in results]).reshape(B, M, H)
    wh_new = np.concatenate([r["wh_new"] for r in results]).reshape(B, M, H)
    acc_p_new = np.concatenate([r["acc_p_new"] for r in results]).reshape(B, M, 1)
    rem_new = np.concatenate([r["rem_new"] for r in results]).reshape(B, M, 1)
    exit_new = np.concatenate([r["exit_new"] for r in results]).reshape(B, M, 1)
    run_new = np.concatenate([r["run_new"] for r in results]).reshape(B, M, 1)
    return (h_out, wh_new, acc_p_new, rem_new, exit_new, run_new)


# revision 48
# speedup vs baseline: 1.1825x; 1.0161x over previous
"""Adaptive Computation Time step — Trainium2 Bass kernel, 8-core data-parallel.

Token layout per core: 8192 tokens -> [128 partitions, 64 columns], token
t = p*64 + c. Each column owns an H=1024 feature row of h. The h@W matvec is
a fused scalar_tensor_tensor (multiply + row-reduce accumulator) per column;
per-token mask/accumulator math is batched [128, G] on VectorE; h_out's mask
multiply runs on ScalarE. All-zero optional inputs (checked on host) skip
their DMA reads; output buffers are pre-zeroed by the runtime.
"""

import numpy as np

import concourse.bacc as bacc
import concourse.mybir as mybir
from concourse.bass_utils import run_bass_kernel_spmd
from concourse.tile import TileContext

N_CORES = 8
P = 128
THRESHOLD = 0.99

# test-harness knobs (the grading harness leaves these at defaults)
TRACE = False
TRACE_CORES = None
LAST_RESULT = None
G_COLS = 4

F32 = mybir.dt.float32
I32 = mybir.dt.int32
OP = mybir.AluOpType
ACTF = mybir.ActivationFunctionType


def build(n_tok, H, G, read_wh, read_rem, read_exit, b_val, coeff_val, step_val):
    """Build the per-core SPMD Bass graph.

    n_tok: tokens per core (multiple of 128). G: columns per DMA group.
    read_*: whether to load that input (host verified it is all-zero otherwise).
    """
    NCOL = n_tok // P
    assert NCOL % G == 0
    NG = NCOL // G

    nc = bacc.Bacc("TRN2", target_bir_lowering=False)

    h_ext = nc.declare_dram_parameter("h", [n_tok, H], F32, isOutput=False)
    wh_ext = (
        nc.declare_dram_parameter("weighted_h", [n_tok, H], F32, isOutput=False)
        if read_wh
        else None
    )
    accp_ext = nc.declare_dram_parameter("acc_p", [n_tok], F32, isOutput=False)
    rem_ext = (
        nc.declare_dram_parameter("remainders", [n_tok], F32, isOutput=False)
        if read_rem
        else None
    )
    exit_ext = (
        nc.declare_dram_parameter("exit_", [n_tok], F32, isOutput=False)
        if read_exit
        else None
    )
    run_ext = nc.declare_dram_parameter("run", [n_tok], I32, isOutput=False)
    w_ext = nc.declare_dram_parameter("W", [H], F32, isOutput=False)

    hout_ext = nc.declare_dram_parameter("h_out", [n_tok, H], F32, isOutput=True)
    whn_ext = nc.declare_dram_parameter("wh_new", [n_tok, H], F32, isOutput=True)
    accn_ext = nc.declare_dram_parameter("acc_p_new", [n_tok], F32, isOutput=True)
    remn_ext = nc.declare_dram_parameter("rem_new", [n_tok], F32, isOutput=True)
    exitn_ext = nc.declare_dram_parameter("exit_new", [n_tok], F32, isOutput=True)
    runn_ext = nc.declare_dram_parameter("run_new", [n_tok], I32, isOutput=True)

    # DRAM views: partition p <- rows p*NCOL..p*NCOL+NCOL-1 (contiguous per
    # partition, so each group DMA moves G*H*4 contiguous bytes per partition).
    hv = h_ext[:, :].rearrange("(p n) d -> p (n d)", p=P)
    whv = wh_ext[:, :].rearrange("(p n) d -> p (n d)", p=P) if read_wh else None
    hov = hout_ext[:, :].rearrange("(p n) d -> p (n d)", p=P)
    whnv = whn_ext[:, :].rearrange("(p n) d -> p (n d)", p=P)

    def small_view(ext):
        return ext[:].rearrange("(p n) -> p n", p=P)

    with TileContext(nc) as tc:
        with (
            tc.tile_pool(name="persist", bufs=1) as persist,
            tc.tile_pool(name="small", bufs=4) as small,
            tc.tile_pool(name="hin", bufs=5) as hpool,
            tc.tile_pool(name="whin", bufs=3) as whpool,
            tc.tile_pool(name="outs", bufs=2) as outpool,
            tc.tile_pool(name="scr", bufs=2) as scratch,
        ):
            # --- persistent loads / per-token precompute ([P, NCOL] each) ---
            # W first (4KB, needed by the first matvec), then the first h tile
            # in halves so compute starts after 1MB instead of 2MB
            wb = persist.tile([P, H], F32)
            nc.sync.dma_start(out=wb[:1, :], in_=w_ext[:].unsqueeze(0))
            nc.gpsimd.partition_broadcast(wb[:], wb[:1, :])

            GH2 = (G * H) // 2
            ht0 = hpool.tile([P, G * H], F32, tag="ht")
            nc.sync.dma_start(out=ht0[:, :GH2], in_=hv[:, 0:GH2])
            nc.sync.dma_start(out=ht0[:, GH2:], in_=hv[:, GH2 : G * H])
            # second tile also ahead of the small loads: keeps the read
            # queue fed through the pipeline-fill phase (prefetch depth 2)
            ht1 = hpool.tile([P, G * H], F32, tag="ht")
            nc.sync.dma_start(out=ht1[:], in_=hv[:, G * H : 2 * G * H])

            accp_sb = persist.tile([P, NCOL], F32)
            nc.sync.dma_start(out=accp_sb[:], in_=small_view(accp_ext))
            runi_sb = persist.tile([P, NCOL], I32)
            nc.sync.dma_start(out=runi_sb[:], in_=small_view(run_ext))
            if read_rem:
                rem_sb = persist.tile([P, NCOL], F32)
                nc.sync.dma_start(out=rem_sb[:], in_=small_view(rem_ext))
            else:
                rem_sb = None
            if read_exit:
                exit_sb = persist.tile([P, NCOL], F32)
                nc.sync.dma_start(out=exit_sb[:], in_=small_view(exit_ext))
            else:
                exit_sb = None

            runof = persist.tile([P, NCOL], F32)  # float(run)
            nc.vector.tensor_copy(runof[:], runi_sb[:])
            runf = persist.tile([P, NCOL], F32)  # (run > 0) as 1.0/0.0
            nc.vector.tensor_scalar(runf[:], runof[:], 0.0, None, OP.is_gt)
            runco = persist.tile([P, NCOL], F32)  # runf * coeff
            nc.vector.tensor_scalar(runco[:], runf[:], float(coeff_val), None, OP.mult)
            q0 = persist.tile([P, NCOL], F32)  # 1 - acc_p
            nc.vector.tensor_scalar(q0[:], accp_sb[:], -1.0, 1.0, OP.mult, OP.add)
            q = persist.tile([P, NCOL], F32)  # runf * (1 - acc_p)
            nc.vector.tensor_tensor(q[:], q0[:], runf[:], OP.mult)

            bias_sb = persist.tile([P, 1], F32)
            nc.vector.memset(bias_sb[:], float(b_val))

            accn_sb = persist.tile([P, NCOL], F32)
            remn_sb = persist.tile([P, NCOL], F32)
            exitn_sb = persist.tile([P, NCOL], F32)
            runnf_sb = persist.tile([P, NCOL], F32)

            if read_wh:
                wt0 = whpool.tile([P, G * H], F32, tag="wt")
                nc.sync.dma_start(out=wt0[:], in_=whv[:, 0 : G * H])
            else:
                wt0 = None

            # --- main loop over column groups (per-token math batched [P, G]).
            # Input DMAs for group g+1 are issued BEFORE group g's output DMAs:
            # the sync engine's stream is in-order, and the output DMAs wait on
            # compute — issuing them first would stall input prefetch behind
            # them (head-of-line blocking on the queue).
            ht, wt = ht0, wt0
            ht_n1 = ht1
            for g in range(NG):
                sl = slice(g * G, (g + 1) * G)
                hot = outpool.tile([P, G * H], F32)
                whnt = outpool.tile([P, G * H], F32, tag="whnt")

                hwg = small.tile([P, G], F32)
                for j in range(G):
                    sc = scratch.tile([P, H], F32)
                    # hw = sum_d h[:,d] * W[d]  (fused multiply + row-reduce)
                    nc.vector.scalar_tensor_tensor(
                        sc[:], ht[:, j * H : (j + 1) * H], 1.0, wb[:],
                        OP.mult, OP.mult, accum_out=hwg[:, j : j + 1],
                    )
                p0g = small.tile([P, G], F32)
                nc.scalar.activation(
                    p0g[:], hwg[:], ACTF.Sigmoid, bias=bias_sb[:], scale=1.0
                )
                pg = small.tile([P, G], F32)  # p = sigmoid * coeff * runf
                nc.vector.tensor_tensor(pg[:], p0g[:], runco[:, sl], OP.mult)
                sg = small.tile([P, G], F32)  # acc_p + p
                nc.vector.tensor_tensor(sg[:], pg[:], accp_sb[:, sl], OP.add)
                mcg = small.tile([P, G], F32)  # (s < T) * runf
                nc.vector.scalar_tensor_tensor(
                    mcg[:], sg[:], THRESHOLD, runf[:, sl], OP.is_lt, OP.mult
                )
                meg = small.tile([P, G], F32)  # runf - mc
                nc.vector.tensor_tensor(meg[:], runf[:, sl], mcg[:], OP.subtract)
                t2g = small.tile([P, G], F32)  # me * (1-acc_p)  (masked)
                nc.vector.tensor_tensor(t2g[:], meg[:], q[:, sl], OP.mult)
                u1g = small.tile([P, G], F32)  # p * mc
                nc.vector.tensor_tensor(u1g[:], mcg[:], pg[:], OP.mult)
                updg = small.tile([P, G], F32)  # p*mc + (1-acc_p)*me
                nc.vector.tensor_tensor(updg[:], u1g[:], t2g[:], OP.add)
                # acc_p_new = acc_p + p*mc
                nc.vector.tensor_tensor(accn_sb[:, sl], u1g[:], accp_sb[:, sl], OP.add)
                # remainders_new = remainders + (1-acc_p)*me
                if read_rem:
                    nc.vector.tensor_tensor(
                        remn_sb[:, sl], t2g[:], rem_sb[:, sl], OP.add
                    )
                else:
                    nc.vector.tensor_copy(remn_sb[:, sl], t2g[:])
                # exit_new = exit_ + step*me
                if read_exit:
                    u2g = small.tile([P, G], F32)
                    nc.vector.tensor_scalar(
                        u2g[:], meg[:], float(step_val), None, OP.mult
                    )
                    nc.vector.tensor_tensor(
                        exitn_sb[:, sl], u2g[:], exit_sb[:, sl], OP.add
                    )
                else:
                    nc.vector.tensor_scalar(
                        exitn_sb[:, sl], meg[:], float(step_val), None, OP.mult
                    )
                # run_new = run * mc (as float; cast at the end)
                nc.vector.tensor_tensor(runnf_sb[:, sl], mcg[:], runof[:, sl], OP.mult)

                for j in range(G):
                    col = g * G + j
                    hs = ht[:, j * H : (j + 1) * H]
                    # weighted_h_new = h*update (+ weighted_h)
                    ws = whnt[:, j * H : (j + 1) * H]
                    if read_wh:
                        nc.vector.scalar_tensor_tensor(
                            ws, hs, updg[:, j : j + 1], wt[:, j * H : (j + 1) * H],
                            OP.mult, OP.add,
                        )
                    else:
                        nc.vector.tensor_scalar(
                            ws, hs, updg[:, j : j + 1], None, OP.mult
                        )
                    # h_out = h * run_new  (ScalarE: per-partition scale)
                    nc.scalar.mul(
                        hot[:, j * H : (j + 1) * H], hs, runnf_sb[:, col : col + 1]
                    )

                ht_n2, wt_next = None, None
                if g + 2 < NG:
                    ht_n2 = hpool.tile([P, G * H], F32, tag="ht")
                    nc.sync.dma_start(
                        out=ht_n2[:],
                        in_=hv[:, (g + 2) * G * H : (g + 3) * G * H],
                    )
                if read_wh and g + 1 < NG:
                    wt_next = whpool.tile([P, G * H], F32, tag="wt")
                    nc.sync.dma_start(
                        out=wt_next[:],
                        in_=whv[:, (g + 1) * G * H : (g + 2) * G * H],
                    )
                # whnt (VectorE, finishes earlier) ahead of hot (ScalarE)
                base = g * G * H
                if g + 1 < NG:
                    nc.sync.dma_start(out=whnv[:, base : base + G * H], in_=whnt[:])
                    nc.sync.dma_start(out=hov[:, base : base + G * H], in_=hot[:])
                else:
                    # last group: halves, interleaved by readiness, so the
                    # final drain starts as soon as the first columns finish
                    nc.sync.dma_start(
                        out=whnv[:, base : base + GH2], in_=whnt[:, :GH2]
                    )
                    nc.sync.dma_start(out=hov[:, base : base + GH2], in_=hot[:, :GH2])
                    nc.sync.dma_start(
                        out=whnv[:, base + GH2 : base + G * H], in_=whnt[:, GH2:]
                    )
                    nc.sync.dma_start(
                        out=hov[:, base + GH2 : base + G * H], in_=hot[:, GH2:]
                    )
                ht, ht_n1, wt = ht_n1, ht_n2, wt_next

            # --- tail: small outputs ---
            runni = persist.tile([P, NCOL], I32)
            nc.vector.tensor_copy(runni[:], runnf_sb[:])
            nc.sync.dma_start(out=small_view(accn_ext), in_=accn_sb[:])
            nc.sync.dma_start(out=small_view(remn_ext), in_=remn_sb[:])
            nc.sync.dma_start(out=small_view(exitn_ext), in_=exitn_sb[:])
            nc.sync.dma_start(out=small_view(runn_ext), in_=runni[:])

    nc.finalize()
    return nc


def kernel(h, weighted_h, acc_p, remainders, exit_, run, W, b, coeff, step):
    h = np.asarray(h, dtype=np.float32)
    weighted_h = np.asarray(weighted_h, dtype=np.float32)
    acc_p = np.asarray(acc_p, dtype=np.float32)
    remainders = np.asarray(remainders, dtype=np.float32)
    exit_ = np.asarray(exit_, dtype=np.float32)
    run = np.asarray(run, dtype=np.int32)
    W = np.asarray(W, dtype=np.float32)
    b_val = float(np.asarray(b, dtype=np.float32).ravel()[0])
    coeff_val = float(np.asarray(coeff, dtype=np.float32).ravel()[0])
    step_val = float(np.asarray(step).ravel()[0])

    B, M, H = h.shape
    n_tok = B * M
    per = n_tok // N_CORES
    read_wh = bool(weighted_h.any())
    read_rem = bool(remainders.any())
    read_exit = bool(exit_.any())

    ncol = per // P
    g = G_COLS if ncol % G_COLS == 0 else (2 if ncol % 2 == 0 else 1)
    nc = build(per, H, g, read_wh, read_rem, read_exit, b_val, coeff_val, step_val)

    hf = h.reshape(n_tok, H)
    whf = weighted_h.reshape(n_tok, H)
    accf = np.ascontiguousarray(acc_p.reshape(n_tok))
    remf = np.ascontiguousarray(remainders.reshape(n_tok))
    exitf = np.ascontiguousarray(exit_.reshape(n_tok))
    runi = np.ascontiguousarray(run.reshape(n_tok))
    wf = np.ascontiguousarray(W.reshape(H))

    in_maps = []
    for c in range(N_CORES):
        s = slice(c * per, (c + 1) * per)
        m = {"h": hf[s], "acc_p": accf[s], "run": runi[s], "W": wf}
        if read_wh:
            m["weighted_h"] = whf[s]
        if read_rem:
            m["remainders"] = remf[s]
        if read_exit:
            m["exit_"] = exitf[s]
        in_maps.append(m)

    global LAST_RESULT
    LAST_RESULT = run_bass_kernel_spmd(
        nc, in_maps, list(range(N_CORES)), trace=TRACE, trace_cores=TRACE_CORES
    )
    results = LAST_RESULT.results

    h_out = np.concatenate([r["h_out"] for r in results]).reshape(B, M, H)
    wh_new = np.concatenate([r["wh_new"] for r in results]).reshape(B, M, H)
    acc_p_new = np.concatenate([r["acc_p_new"] for r in results]).reshape(B, M, 1)
    rem_new = np.concatenate([r["rem_new"] for r in results]).reshape(B, M, 1)
    exit_new = np.concatenate([r["exit_new"] for r in results]).reshape(B, M, 1)
    run_new = np.concatenate([r["run_new"] for r in results]).reshape(B, M, 1)
    return (h_out, wh_new, acc_p_new, rem_new, exit_new, run_new)
